# revision 11
# baseline (speedup 1.0000x reference)
"""Trainium2 Bass kernel for nn_Normalizer (annealed top-k masking normalizer).

Math (see reference): the T=20 annealed-theta loop converges; the output
depends only on the fixed point c* of  s(c) = k*c  where, in exp space,
E = exp(masked_score/theta),  s(c) = sum_j min(E_j, c),  k = 0.1 * n_finite.
The scheduled trajectory's c_19 differs from c* by ~1e-4 relative, far
below the accuracy gate, so the kernel solves the fixed point directly:

  1. host: sm = fp16(score, masked -> -60000)           [halves input DMA]
  2. ACT:  E = bf16(exp(sm/theta)) per 128-row tile, in column halves so
     compute starts as soon as the first half-DMA lands
  3. k = 0.1 * count(finite): DVE slice counts sm > -1000 (runs during the
     exp phase, straight off the fp16 input); ACT slice counts via
     sum(relu(1 - HUGE*E)) = width - count (exact: masked E is exactly 0);
     the ACT count slices are emitted after the sub phase so they fill ACT
     idle gaps -- they are only needed by the first full-width update.
  4. sub phase: 5 fixed-point iters on a 1/16 column subsample
     (8 cols every 128) read directly from E via a strided view;
     tiles {0,1} iterate on DVE, tiles {2,3} on ACT (iter0:
     Identity+accum = plain sum; then relu-trick) with their scalar
     updates on the otherwise-idle Pool engine.
  5. full phase "FSS": three full-width s(c) passes, column-sliced across
     DVE (min+accum) and ACT (relu-trick: sum min = W*c - sum relu(c-E));
     update 1 = plain fixed point c=s/k, updates 2,3 = secant (slope from
     the last two (c, s) pairs) -- no full count passes needed.  The
     c-only secant inputs (dc, dc2, rdc, kc) are computed during the
     s-pass on Pool/DVE so the post-pass critical chain is short.
  6. gamma = bf16(min(E * (1/c), 1)) in place over E (DVE 4x mode, in
     halves), DMA out as bf16; host upcasts to f32.

All row-scalars for the 4 tiles are batched as [128,4] (or per-group
[128,2]) f32 tiles so each scalar update is one instruction per core.
The Pool engine only supports tensor_tensor add/mult/sub + memset in this
toolchain, so it gets exactly those.

Sharding: pure row-parallel, 4096 rows -> 8 cores x 512 rows.
"""

import os
import sys

import numpy as np

try:
    import concourse.bass as bass  # noqa: F401
except ImportError:
    sys.path.insert(0, "/opt/trn_rl_repo")
    import concourse.bass as bass  # noqa: F401

import ml_dtypes  # noqa: F401

import concourse.bacc as bacc
import concourse.tile as tile
from concourse import mybir
from concourse.bass_utils import run_bass_kernel_spmd

F32 = mybir.dt.float32
BF16 = mybir.dt.bfloat16
FP16 = mybir.dt.float16
A = mybir.AluOpType
AF = mybir.ActivationFunctionType

THETA, P_FRAC = 0.3, 0.1
BSZ, SEQ = 4096, 8192
N_CORES = 8
ROWS_PER_CORE = BSZ // N_CORES          # 512
P = 128
N_TILES = ROWS_PER_CORE // P            # 4
HALF = SEQ // 2
CHUNK, CHUNK_EVERY = 8, 128             # subsample: 8 cols every 128
SUB = SEQ // CHUNK_EVERY * CHUNK        # 512
BIG = 1.0e30
HH = 1.0e25                             # relu count scaling
MASKVAL = -60000.0                      # fp16-representable, exp -> 0
SM_THRESH = -1000.0                     # finite iff sm > this

N_SUB = int(os.environ.get("NORM_SUB_ITERS", "5"))
FULL_SEQ = os.environ.get("NORM_FULL_SEQ", "FSS")  # F=fixed point, S=secant
# column-slice widths (DVE vs ACT) for the full s-passes and k-passes
S_DVE = int(os.environ.get("NORM_S_DVE", "3712"))
S_ACT = SEQ - S_DVE
K_DVE = int(os.environ.get("NORM_K_DVE", "4096"))
K_ACT = SEQ - K_DVE
DEBUG = os.environ.get("NORM_DEBUG", "0") == "1"


def _sub_view(ap):
    """[P, SEQ] AP -> [P, 64, CHUNK] strided subsample view."""
    return ap.rearrange("p (c l) -> p c l", l=CHUNK_EVERY)[:, :, 0:CHUNK]


def _sub_out(ap):
    """[P, SUB] contiguous AP -> [P, 64, CHUNK] view."""
    return ap.rearrange("p (c l) -> p c l", l=CHUNK)


def build_kernel():
    nc = bacc.Bacc("TRN2", target_bir_lowering=False, debug=False,
                   num_devices=N_CORES)
    sm_d = nc.dram_tensor("sm", [ROWS_PER_CORE, SEQ], FP16,
                          kind="ExternalInput")
    gamma_d = nc.dram_tensor("gamma", [ROWS_PER_CORE, SEQ], BF16,
                             kind="ExternalOutput")

    v = nc.vector
    g = nc.gpsimd
    s = nc.scalar

    with tile.TileContext(nc) as tc:
        with (
            tc.tile_pool(name="smp", bufs=1) as smp,
            tc.tile_pool(name="ep", bufs=1) as ep,
            tc.tile_pool(name="jdp", bufs=1) as jdp,
            tc.tile_pool(name="jap", bufs=1) as jap,
            tc.tile_pool(name="jsp", bufs=1) as jsp,
            tc.tile_pool(name="scal", bufs=8) as scal,
        ):
            jD = jdp.tile([P, max(K_DVE, S_DVE, SUB)], F32, tag="jD")
            jA = jap.tile([P, max(K_ACT, S_ACT)], F32, tag="jA")
            jSD = jsp.tile([P, SUB], F32, tag="jSD")
            jSA = jsp.tile([P, SUB], F32, tag="jSA")

            cnts4 = scal.tile([P, N_TILES], F32, tag="cnts")
            kD4 = scal.tile([P, N_TILES], F32, tag="kD")
            rkA4 = scal.tile([P, N_TILES], F32, tag="rkA")
            cSUB = scal.tile([P, 2], F32, tag="cSUB")
            g.memset(cSUB[:], float(SUB))
            eps30 = scal.tile([P, N_TILES], F32, tag="eps30")
            g.memset(eps30[:], 1e-30)

            # ---- phase A: DMA in, exp, DVE count slices -----------------
            E = []
            for j in range(N_TILES):
                r0 = j * P
                sm = smp.tile([P, SEQ], FP16, tag=f"sm{j % 2}")
                nc.sync.dma_start(out=sm[:][:, 0:HALF],
                                  in_=sm_d.ap()[r0:r0 + P, 0:HALF])
                nc.sync.dma_start(out=sm[:][:, HALF:SEQ],
                                  in_=sm_d.ap()[r0:r0 + P, HALF:SEQ])
                e_t = ep.tile([P, SEQ], BF16, tag=f"E{j}")
                E.append(e_t)
                s.activation(out=e_t[:][:, 0:HALF], in_=sm[:][:, 0:HALF],
                             func=AF.Exp, scale=1.0 / THETA)
                s.activation(out=e_t[:][:, HALF:SEQ], in_=sm[:][:, HALF:SEQ],
                             func=AF.Exp, scale=1.0 / THETA)
                # full-count DVE slice off sm
                v.tensor_scalar(out=jD[:][:, 0:K_DVE],
                                in0=sm[:][:, 0:K_DVE],
                                scalar1=SM_THRESH, scalar2=None,
                                op0=A.is_gt, op1=A.add,
                                accum_out=kD4[:, j:j + 1])
                # subsample count off sm
                v.tensor_scalar(out=_sub_out(jSD[:]), in0=_sub_view(sm[:]),
                                scalar1=SM_THRESH, scalar2=None,
                                op0=A.is_gt, op1=A.add,
                                accum_out=cnts4[:, j:j + 1])

            # per-group subsample rks = 10 / cnt_sub
            rks = []
            for grp in range(2):
                rc_ = scal.tile([P, 2], F32, tag=f"rcs{grp}")
                v.reciprocal(rc_[:], cnts4[:, 2 * grp:2 * grp + 2])
                rk_ = scal.tile([P, 2], F32, tag=f"rks{grp}")
                v.tensor_scalar_mul(rk_[:], rc_[:], 1.0 / P_FRAC)
                rks.append(rk_)

            # ---- phase B: subsample fixed point, 2 chains ---------------
            # group A = tiles {0,1} on DVE, group B = tiles {2,3} on ACT
            cA = cB = None
            for it in range(N_SUB):
                sA = scal.tile([P, 2], F32, tag="sgA")
                rB = scal.tile([P, 2], F32, tag="sgB")
                for jj in range(2):
                    v.tensor_scalar(out=_sub_out(jSD[:]),
                                    in0=_sub_view(E[jj][:]),
                                    scalar1=(BIG if it == 0
                                             else cA[:, jj:jj + 1]),
                                    scalar2=None,
                                    op0=A.min, op1=A.add,
                                    accum_out=sA[:, jj:jj + 1])
                for jj in range(2):
                    if it == 0:
                        s.activation(out=_sub_out(jSA[:]),
                                     in_=_sub_view(E[2 + jj][:]),
                                     func=AF.Identity,
                                     accum_out=rB[:, jj:jj + 1])
                    else:
                        s.activation(out=_sub_out(jSA[:]),
                                     in_=_sub_view(E[2 + jj][:]),
                                     func=AF.Relu, scale=-1.0,
                                     bias=cB[:, jj:jj + 1],
                                     accum_out=rB[:, jj:jj + 1])
                cAn = scal.tile([P, 2], F32, tag="cgA")
                v.tensor_mul(cAn[:], sA[:], rks[0][:])
                cBn = scal.tile([P, 2], F32, tag="cgB")
                if it == 0:
                    g.tensor_mul(cBn[:], rB[:], rks[1][:])
                else:
                    # s = SUB*c - r ; c' = s * rks   (all on Pool)
                    uB = scal.tile([P, 2], F32, tag="ugB")
                    g.tensor_mul(uB[:], cB[:], cSUB[:])
                    tB = scal.tile([P, 2], F32, tag="tgB")
                    g.tensor_sub(tB[:], uB[:], rB[:])
                    g.tensor_mul(cBn[:], tB[:], rks[1][:])
                cA, cB = cAn, cBn

            # merge group c into batched [P,4]
            c4 = scal.tile([P, N_TILES], F32, tag="c4m")
            v.tensor_copy(c4[:, 0:2], cA[:])
            v.tensor_copy(c4[:, 2:4], cB[:])

            # ---- ACT count slices (fill ACT gaps) + k prep --------------
            for j in range(N_TILES):
                s.activation(out=jA[:][:, 0:K_ACT],
                             in_=E[j][:][:, K_DVE:SEQ],
                             func=AF.Relu, scale=-HH, bias=1.0,
                             accum_out=rkA4[:, j:j + 1])
            t1 = scal.tile([P, N_TILES], F32, tag="t1")
            v.scalar_tensor_tensor(out=t1[:], in0=rkA4[:], scalar=-1.0,
                                   in1=kD4[:], op0=A.mult, op1=A.add)
            cnt4 = scal.tile([P, N_TILES], F32, tag="cnt4")
            v.tensor_scalar_add(cnt4[:], t1[:], float(K_ACT))
            k4 = scal.tile([P, N_TILES], F32, tag="k4")
            v.tensor_scalar_mul(k4[:], cnt4[:], P_FRAC)
            rk4 = scal.tile([P, N_TILES], F32, tag="rk4")
            v.reciprocal(rk4[:], k4[:])
            k02 = scal.tile([P, N_TILES], F32, tag="k02")
            v.tensor_scalar_mul(k02[:], k4[:], 0.02)

            # ---- phase C: full-width passes (FSS) -----------------------
            def full_s_pass(c_t, tag):
                sD = scal.tile([P, N_TILES], F32, tag="sD" + tag)
                rA = scal.tile([P, N_TILES], F32, tag="rA" + tag)
                for j in range(N_TILES):
                    cj = c_t[:, j:j + 1]
                    v.tensor_scalar(out=jD[:][:, 0:S_DVE],
                                    in0=E[j][:][:, 0:S_DVE],
                                    scalar1=cj, scalar2=None,
                                    op0=A.min, op1=A.add,
                                    accum_out=sD[:, j:j + 1])
                    s.activation(out=jA[:][:, 0:S_ACT],
                                 in_=E[j][:][:, S_DVE:SEQ],
                                 func=AF.Relu, scale=-1.0, bias=cj,
                                 accum_out=rA[:, j:j + 1])
                # s = sD + S_ACT*c - rA
                u1 = scal.tile([P, N_TILES], F32, tag="u1" + tag)
                v.scalar_tensor_tensor(out=u1[:], in0=c_t[:],
                                       scalar=float(S_ACT), in1=rA[:],
                                       op0=A.mult, op1=A.subtract)
                s4 = scal.tile([P, N_TILES], F32, tag="s4" + tag)
                v.tensor_add(s4[:], sD[:], u1[:])
                return s4

            cp, sp_ = None, None
            for i, stepc in enumerate(FULL_SEQ):
                if stepc != "F":
                    # c-only secant inputs: run during the s-pass
                    dc = scal.tile([P, N_TILES], F32, tag=f"dc{i}")
                    g.tensor_sub(dc[:], c4[:], cp[:])
                    ec = scal.tile([P, N_TILES], F32, tag=f"ec{i}")
                    g.tensor_mul(ec[:], c4[:], eps30[:])
                    dc2 = scal.tile([P, N_TILES], F32, tag=f"dc2{i}")
                    g.tensor_add(dc2[:], dc[:], ec[:])
                    kc = scal.tile([P, N_TILES], F32, tag=f"kc{i}")
                    g.tensor_mul(kc[:], k4[:], c4[:])
                    rdc = scal.tile([P, N_TILES], F32, tag=f"rdc{i}")
                    v.reciprocal(rdc[:], dc2[:])
                s4 = full_s_pass(c4, f"f{i}")
                cn = scal.tile([P, N_TILES], F32, tag=f"c4_{i}")
                if stepc == "F":
                    v.tensor_mul(cn[:], s4[:], rk4[:])
                else:  # secant, post-pass chain
                    ds = scal.tile([P, N_TILES], F32, tag=f"ds{i}")
                    v.tensor_sub(ds[:], s4[:], sp_[:])
                    m_ = scal.tile([P, N_TILES], F32, tag=f"m{i}")
                    v.tensor_mul(m_[:], ds[:], rdc[:])
                    den = scal.tile([P, N_TILES], F32, tag=f"den{i}")
                    v.tensor_sub(den[:], k4[:], m_[:])
                    den2 = scal.tile([P, N_TILES], F32, tag=f"den2{i}")
                    v.tensor_max(den2[:], den[:], k02[:])
                    rden = scal.tile([P, N_TILES], F32, tag=f"rden{i}")
                    v.reciprocal(rden[:], den2[:])
                    num = scal.tile([P, N_TILES], F32, tag=f"num{i}")
                    v.tensor_sub(num[:], s4[:], kc[:])
                    tq = scal.tile([P, N_TILES], F32, tag=f"tq{i}")
                    v.tensor_mul(tq[:], num[:], rden[:])
                    v.tensor_add(cn[:], c4[:], tq[:])
                cp, sp_ = c4, s4
                c4 = cn

            # ---- phase D: gamma (in place over E, halves), DMA out ------
            rc4 = scal.tile([P, N_TILES], F32, tag="rc4")
            v.reciprocal(rc4[:], c4[:])
            for j in range(N_TILES):
                r0 = j * P
                for h0, h1 in ((0, HALF), (HALF, SEQ)):
                    v.tensor_scalar(out=E[j][:][:, h0:h1],
                                    in0=E[j][:][:, h0:h1],
                                    scalar1=rc4[:, j:j + 1], scalar2=1.0,
                                    op0=A.mult, op1=A.min)
                    nc.sync.dma_start(out=gamma_d.ap()[r0:r0 + P, h0:h1],
                                      in_=E[j][:][:, h0:h1])

    nc.compile()
    return nc


_NC_CACHE = None


def prep_sm(score: np.ndarray, mask: np.ndarray) -> np.ndarray:
    """host-side dtype prep: masked score in fp16 (elementwise only)."""
    return np.where(np.asarray(mask) == 0, np.float16(MASKVAL),
                    np.asarray(score).astype(np.float16))


def kernel(score: np.ndarray, mask: np.ndarray) -> np.ndarray:
    global _NC_CACHE
    if _NC_CACHE is None:
        _NC_CACHE = build_kernel()
    nc = _NC_CACHE

    sm16 = np.ascontiguousarray(prep_sm(score, mask))
    in_maps = []
    for i in range(N_CORES):
        sl = slice(i * ROWS_PER_CORE, (i + 1) * ROWS_PER_CORE)
        in_maps.append({"sm": sm16[sl]})
    res = run_bass_kernel_spmd(nc, in_maps, core_ids=list(range(N_CORES)))
    out = np.concatenate([res.results[i]["gamma"] for i in range(N_CORES)],
                         axis=0)
    return out.astype(np.float32)


# revision 12
# speedup vs baseline: 1.0434x; 1.0434x over previous
"""Trainium2 Bass kernel for nn_Normalizer (annealed top-k masking normalizer).

Math (see reference): the T=20 annealed-theta loop converges; the output
depends only on the fixed point c* of  s(c) = k*c  where, in exp space,
E = exp(masked_score/theta),  s(c) = sum_j min(E_j, c),  k = 0.1 * n_finite.
The scheduled trajectory's c_19 differs from c* by ~1e-4 relative, far
below the accuracy gate, so the kernel solves the fixed point directly:

  1. host: sm = fp16(score, masked -> -60000)           [halves input DMA]
  2. ACT:  E = bf16(exp(sm/theta)) per 128-row tile, in column halves so
     compute starts as soon as the first half-DMA lands
  3. k = 0.1 * count(finite): DVE slice counts sm > -1000 (runs during the
     exp phase, straight off the fp16 input); ACT slice counts via
     sum(relu(1 - HUGE*E)) = width - count (exact: masked E is exactly 0);
     the ACT count slices are emitted after the sub phase so they fill ACT
     idle gaps -- they are only needed by the first full-width update.
  4. sub phase: 5 fixed-point iters on a 1/16 column subsample
     (8 cols every 128) read directly from E via a strided view;
     tiles {0,1} iterate on DVE, tiles {2,3} on ACT (iter0:
     Identity+accum = plain sum; then relu-trick) with their scalar
     updates on the otherwise-idle Pool engine.
  5. full phase "FSS": three full-width s(c) passes, column-sliced across
     DVE (min+accum) and ACT (relu-trick: sum min = W*c - sum relu(c-E));
     update 1 = plain fixed point c=s/k, updates 2,3 = secant (slope from
     the last two (c, s) pairs) -- no full count passes needed.  The
     c-only secant inputs (dc, dc2, rdc, kc) are computed during the
     s-pass on Pool/DVE so the post-pass critical chain is short.
  6. gamma = bf16(min(E * (1/c), 1)) in place over E (DVE 4x mode, in
     halves), DMA out as bf16; host upcasts to f32.

All row-scalars for the 4 tiles are batched as [128,4] (or per-group
[128,2]) f32 tiles so each scalar update is one instruction per core.
The Pool engine only supports tensor_tensor add/mult/sub + memset in this
toolchain, so it gets exactly those.

Sharding: pure row-parallel, 4096 rows -> 8 cores x 512 rows.
"""

import os
import sys

import numpy as np

try:
    import concourse.bass as bass  # noqa: F401
except ImportError:
    sys.path.insert(0, "/opt/trn_rl_repo")
    import concourse.bass as bass  # noqa: F401

import ml_dtypes  # noqa: F401

import concourse.bacc as bacc
import concourse.tile as tile
from concourse import mybir
from concourse.bass_utils import run_bass_kernel_spmd

F32 = mybir.dt.float32
BF16 = mybir.dt.bfloat16
FP16 = mybir.dt.float16
A = mybir.AluOpType
AF = mybir.ActivationFunctionType

THETA, P_FRAC = 0.3, 0.1
BSZ, SEQ = 4096, 8192
N_CORES = 8
ROWS_PER_CORE = BSZ // N_CORES          # 512
P = 128
N_TILES = ROWS_PER_CORE // P            # 4
HALF = SEQ // 2
CHUNK, CHUNK_EVERY = 8, 128             # subsample: 8 cols every 128
SUB = SEQ // CHUNK_EVERY * CHUNK        # 512
BIG = 1.0e30
HH = 1.0e25                             # relu count scaling
MASKVAL = -60000.0                      # fp16-representable, exp -> 0
SM_THRESH = -1000.0                     # finite iff sm > this

N_SUB = int(os.environ.get("NORM_SUB_ITERS", "5"))
FULL_SEQ = os.environ.get("NORM_FULL_SEQ", "FSS")  # F=fixed point, S=secant
# column-slice widths (DVE vs ACT) for the full s-passes and k-passes
S_DVE = int(os.environ.get("NORM_S_DVE", "3712"))
S_ACT = SEQ - S_DVE
K_DVE = int(os.environ.get("NORM_K_DVE", "4096"))
K_ACT = SEQ - K_DVE
DEBUG = os.environ.get("NORM_DEBUG", "0") == "1"


def _sub_view(ap):
    """[P, SEQ] AP -> [P, 64, CHUNK] strided subsample view."""
    return ap.rearrange("p (c l) -> p c l", l=CHUNK_EVERY)[:, :, 0:CHUNK]


def _sub_out(ap):
    """[P, SUB] contiguous AP -> [P, 64, CHUNK] view."""
    return ap.rearrange("p (c l) -> p c l", l=CHUNK)


def build_kernel():
    nc = bacc.Bacc("TRN2", target_bir_lowering=False, debug=False,
                   num_devices=N_CORES)
    sm_d = nc.dram_tensor("sm", [ROWS_PER_CORE, SEQ], FP16,
                          kind="ExternalInput")
    gamma_d = nc.dram_tensor("gamma", [ROWS_PER_CORE, SEQ], BF16,
                             kind="ExternalOutput")

    v = nc.vector
    g = nc.gpsimd
    s = nc.scalar

    with tile.TileContext(nc) as tc:
        with (
            tc.tile_pool(name="smp", bufs=1) as smp,
            tc.tile_pool(name="ep", bufs=1) as ep,
            tc.tile_pool(name="jdp", bufs=1) as jdp,
            tc.tile_pool(name="jap", bufs=1) as jap,
            tc.tile_pool(name="jsp", bufs=1) as jsp,
            tc.tile_pool(name="scal", bufs=8) as scal,
        ):
            jD = jdp.tile([P, max(K_DVE, S_DVE, SUB)], F32, tag="jD")
            jA = jap.tile([P, max(K_ACT, S_ACT)], F32, tag="jA")
            jSD = jsp.tile([P, SUB], F32, tag="jSD")
            jSA = jsp.tile([P, SUB], F32, tag="jSA")

            cnts4 = scal.tile([P, N_TILES], F32, tag="cnts")
            kD4 = scal.tile([P, N_TILES], F32, tag="kD")
            rkA4 = scal.tile([P, N_TILES], F32, tag="rkA")
            cSUB = scal.tile([P, 2], F32, tag="cSUB")
            g.memset(cSUB[:], float(SUB))
            eps30 = scal.tile([P, N_TILES], F32, tag="eps30")
            g.memset(eps30[:], 1e-30)

            # ---- phase A: DMA in, exp, DVE count slices -----------------
            E = []
            for j in range(N_TILES):
                r0 = j * P
                sm = smp.tile([P, SEQ], FP16, tag=f"sm{j}")
                nc.sync.dma_start(out=sm[:][:, 0:HALF],
                                  in_=sm_d.ap()[r0:r0 + P, 0:HALF])
                nc.sync.dma_start(out=sm[:][:, HALF:SEQ],
                                  in_=sm_d.ap()[r0:r0 + P, HALF:SEQ])
                e_t = ep.tile([P, SEQ], BF16, tag=f"E{j}")
                E.append(e_t)
                s.activation(out=e_t[:][:, 0:HALF], in_=sm[:][:, 0:HALF],
                             func=AF.Exp, scale=1.0 / THETA)
                s.activation(out=e_t[:][:, HALF:SEQ], in_=sm[:][:, HALF:SEQ],
                             func=AF.Exp, scale=1.0 / THETA)
                # full-count DVE slice off sm
                v.tensor_scalar(out=jD[:][:, 0:K_DVE],
                                in0=sm[:][:, 0:K_DVE],
                                scalar1=SM_THRESH, scalar2=None,
                                op0=A.is_gt, op1=A.add,
                                accum_out=kD4[:, j:j + 1])
                # subsample count off sm
                v.tensor_scalar(out=_sub_out(jSD[:]), in0=_sub_view(sm[:]),
                                scalar1=SM_THRESH, scalar2=None,
                                op0=A.is_gt, op1=A.add,
                                accum_out=cnts4[:, j:j + 1])

            # per-group subsample rks = 10 / cnt_sub
            rks = []
            for grp in range(2):
                rc_ = scal.tile([P, 2], F32, tag=f"rcs{grp}")
                v.reciprocal(rc_[:], cnts4[:, 2 * grp:2 * grp + 2])
                rk_ = scal.tile([P, 2], F32, tag=f"rks{grp}")
                v.tensor_scalar_mul(rk_[:], rc_[:], 1.0 / P_FRAC)
                rks.append(rk_)

            # ---- phase B: subsample fixed point, 2 chains ---------------
            # group A = tiles {0,1} on DVE, group B = tiles {2,3} on ACT
            cA = cB = None
            for it in range(N_SUB):
                sA = scal.tile([P, 2], F32, tag="sgA")
                rB = scal.tile([P, 2], F32, tag="sgB")
                for jj in range(2):
                    v.tensor_scalar(out=_sub_out(jSD[:]),
                                    in0=_sub_view(E[jj][:]),
                                    scalar1=(BIG if it == 0
                                             else cA[:, jj:jj + 1]),
                                    scalar2=None,
                                    op0=A.min, op1=A.add,
                                    accum_out=sA[:, jj:jj + 1])
                for jj in range(2):
                    if it == 0:
                        s.activation(out=_sub_out(jSA[:]),
                                     in_=_sub_view(E[2 + jj][:]),
                                     func=AF.Identity,
                                     accum_out=rB[:, jj:jj + 1])
                    else:
                        s.activation(out=_sub_out(jSA[:]),
                                     in_=_sub_view(E[2 + jj][:]),
                                     func=AF.Relu, scale=-1.0,
                                     bias=cB[:, jj:jj + 1],
                                     accum_out=rB[:, jj:jj + 1])
                cAn = scal.tile([P, 2], F32, tag="cgA")
                v.tensor_mul(cAn[:], sA[:], rks[0][:])
                cBn = scal.tile([P, 2], F32, tag="cgB")
                if it == 0:
                    g.tensor_mul(cBn[:], rB[:], rks[1][:])
                else:
                    # s = SUB*c - r ; c' = s * rks   (all on Pool)
                    uB = scal.tile([P, 2], F32, tag="ugB")
                    g.tensor_mul(uB[:], cB[:], cSUB[:])
                    tB = scal.tile([P, 2], F32, tag="tgB")
                    g.tensor_sub(tB[:], uB[:], rB[:])
                    g.tensor_mul(cBn[:], tB[:], rks[1][:])
                cA, cB = cAn, cBn

            # merge group c into batched [P,4]
            c4 = scal.tile([P, N_TILES], F32, tag="c4m")
            v.tensor_copy(c4[:, 0:2], cA[:])
            v.tensor_copy(c4[:, 2:4], cB[:])

            # ---- ACT count slices (fill ACT gaps) + k prep --------------
            for j in range(N_TILES):
                s.activation(out=jA[:][:, 0:K_ACT],
                             in_=E[j][:][:, K_DVE:SEQ],
                             func=AF.Relu, scale=-HH, bias=1.0,
                             accum_out=rkA4[:, j:j + 1])
            t1 = scal.tile([P, N_TILES], F32, tag="t1")
            v.scalar_tensor_tensor(out=t1[:], in0=rkA4[:], scalar=-1.0,
                                   in1=kD4[:], op0=A.mult, op1=A.add)
            cnt4 = scal.tile([P, N_TILES], F32, tag="cnt4")
            v.tensor_scalar_add(cnt4[:], t1[:], float(K_ACT))
            k4 = scal.tile([P, N_TILES], F32, tag="k4")
            v.tensor_scalar_mul(k4[:], cnt4[:], P_FRAC)
            rk4 = scal.tile([P, N_TILES], F32, tag="rk4")
            v.reciprocal(rk4[:], k4[:])
            k02 = scal.tile([P, N_TILES], F32, tag="k02")
            v.tensor_scalar_mul(k02[:], k4[:], 0.02)

            # ---- phase C: full-width passes (FSS) -----------------------
            def full_s_pass(c_t, tag):
                sD = scal.tile([P, N_TILES], F32, tag="sD" + tag)
                rA = scal.tile([P, N_TILES], F32, tag="rA" + tag)
                for j in range(N_TILES):
                    cj = c_t[:, j:j + 1]
                    v.tensor_scalar(out=jD[:][:, 0:S_DVE],
                                    in0=E[j][:][:, 0:S_DVE],
                                    scalar1=cj, scalar2=None,
                                    op0=A.min, op1=A.add,
                                    accum_out=sD[:, j:j + 1])
                    s.activation(out=jA[:][:, 0:S_ACT],
                                 in_=E[j][:][:, S_DVE:SEQ],
                                 func=AF.Relu, scale=-1.0, bias=cj,
                                 accum_out=rA[:, j:j + 1])
                # s = sD + S_ACT*c - rA
                u1 = scal.tile([P, N_TILES], F32, tag="u1" + tag)
                v.scalar_tensor_tensor(out=u1[:], in0=c_t[:],
                                       scalar=float(S_ACT), in1=rA[:],
                                       op0=A.mult, op1=A.subtract)
                s4 = scal.tile([P, N_TILES], F32, tag="s4" + tag)
                v.tensor_add(s4[:], sD[:], u1[:])
                return s4

            cp, sp_ = None, None
            for i, stepc in enumerate(FULL_SEQ):
                if stepc != "F":
                    # c-only secant inputs: run during the s-pass
                    dc = scal.tile([P, N_TILES], F32, tag=f"dc{i}")
                    g.tensor_sub(dc[:], c4[:], cp[:])
                    ec = scal.tile([P, N_TILES], F32, tag=f"ec{i}")
                    g.tensor_mul(ec[:], c4[:], eps30[:])
                    dc2 = scal.tile([P, N_TILES], F32, tag=f"dc2{i}")
                    g.tensor_add(dc2[:], dc[:], ec[:])
                    kc = scal.tile([P, N_TILES], F32, tag=f"kc{i}")
                    g.tensor_mul(kc[:], k4[:], c4[:])
                    rdc = scal.tile([P, N_TILES], F32, tag=f"rdc{i}")
                    v.reciprocal(rdc[:], dc2[:])
                s4 = full_s_pass(c4, f"f{i}")
                cn = scal.tile([P, N_TILES], F32, tag=f"c4_{i}")
                if stepc == "F":
                    v.tensor_mul(cn[:], s4[:], rk4[:])
                else:  # secant, post-pass chain
                    ds = scal.tile([P, N_TILES], F32, tag=f"ds{i}")
                    v.tensor_sub(ds[:], s4[:], sp_[:])
                    m_ = scal.tile([P, N_TILES], F32, tag=f"m{i}")
                    v.tensor_mul(m_[:], ds[:], rdc[:])
                    den = scal.tile([P, N_TILES], F32, tag=f"den{i}")
                    v.tensor_sub(den[:], k4[:], m_[:])
                    den2 = scal.tile([P, N_TILES], F32, tag=f"den2{i}")
                    v.tensor_max(den2[:], den[:], k02[:])
                    rden = scal.tile([P, N_TILES], F32, tag=f"rden{i}")
                    v.reciprocal(rden[:], den2[:])
                    num = scal.tile([P, N_TILES], F32, tag=f"num{i}")
                    v.tensor_sub(num[:], s4[:], kc[:])
                    tq = scal.tile([P, N_TILES], F32, tag=f"tq{i}")
                    v.tensor_mul(tq[:], num[:], rden[:])
                    v.tensor_add(cn[:], c4[:], tq[:])
                cp, sp_ = c4, s4
                c4 = cn

            # ---- phase D: gamma (in place over E, halves), DMA out ------
            rc4 = scal.tile([P, N_TILES], F32, tag="rc4")
            v.reciprocal(rc4[:], c4[:])
            for j in range(N_TILES):
                r0 = j * P
                for h0, h1 in ((0, HALF), (HALF, SEQ)):
                    v.tensor_scalar(out=E[j][:][:, h0:h1],
                                    in0=E[j][:][:, h0:h1],
                                    scalar1=rc4[:, j:j + 1], scalar2=1.0,
                                    op0=A.mult, op1=A.min)
                    nc.sync.dma_start(out=gamma_d.ap()[r0:r0 + P, h0:h1],
                                      in_=E[j][:][:, h0:h1])

    nc.compile()
    return nc


_NC_CACHE = None


def prep_sm(score: np.ndarray, mask: np.ndarray) -> np.ndarray:
    """host-side dtype prep: masked score in fp16 (elementwise only)."""
    return np.where(np.asarray(mask) == 0, np.float16(MASKVAL),
                    np.asarray(score).astype(np.float16))


def kernel(score: np.ndarray, mask: np.ndarray) -> np.ndarray:
    global _NC_CACHE
    if _NC_CACHE is None:
        _NC_CACHE = build_kernel()
    nc = _NC_CACHE

    sm16 = np.ascontiguousarray(prep_sm(score, mask))
    in_maps = []
    for i in range(N_CORES):
        sl = slice(i * ROWS_PER_CORE, (i + 1) * ROWS_PER_CORE)
        in_maps.append({"sm": sm16[sl]})
    res = run_bass_kernel_spmd(nc, in_maps, core_ids=list(range(N_CORES)))
    out = np.concatenate([res.results[i]["gamma"] for i in range(N_CORES)],
                         axis=0)
    return out.astype(np.float32)


# revision 13
# speedup vs baseline: 1.0468x; 1.0032x over previous
"""Trainium2 Bass kernel for nn_Normalizer (annealed top-k masking normalizer).

Math (see reference): the T=20 annealed-theta loop converges; the output
depends only on the fixed point c* of  s(c) = k*c  where, in exp space,
E = exp(masked_score/theta),  s(c) = sum_j min(E_j, c),  k = 0.1 * n_finite.
The scheduled trajectory's c_19 differs from c* by ~1e-4 relative, far
below the accuracy gate, so the kernel solves the fixed point directly:

  1. host: sm = fp16(score, masked -> -60000)           [halves input DMA]
  2. ACT:  E = bf16(exp(sm/theta)) per 128-row tile, in column halves so
     compute starts as soon as the first half-DMA lands
  3. k = 0.1 * count(finite): DVE slice counts sm > -1000 (runs during the
     exp phase, straight off the fp16 input); ACT slice counts via
     sum(relu(1 - HUGE*E)) = width - count (exact: masked E is exactly 0);
     the ACT count slices are emitted after the sub phase so they fill ACT
     idle gaps -- they are only needed by the first full-width update.
  4. sub phase: 5 fixed-point iters on a 1/16 column subsample
     (8 cols every 128) read directly from E via a strided view;
     tiles {0,1} iterate on DVE, tiles {2,3} on ACT (iter0:
     Identity+accum = plain sum; then relu-trick) with their scalar
     updates on the otherwise-idle Pool engine.
  5. full phase "FSS": three full-width s(c) passes, column-sliced across
     DVE (min+accum) and ACT (relu-trick: sum min = W*c - sum relu(c-E));
     update 1 = plain fixed point c=s/k, updates 2,3 = secant (slope from
     the last two (c, s) pairs) -- no full count passes needed.  The
     c-only secant inputs (dc, dc2, rdc, kc) are computed during the
     s-pass on Pool/DVE so the post-pass critical chain is short.
  6. gamma = bf16(min(E * (1/c), 1)) in place over E (DVE 4x mode, in
     halves), DMA out as bf16; host upcasts to f32.

All row-scalars for the 4 tiles are batched as [128,4] (or per-group
[128,2]) f32 tiles so each scalar update is one instruction per core.
The Pool engine only supports tensor_tensor add/mult/sub + memset in this
toolchain, so it gets exactly those.

Sharding: pure row-parallel, 4096 rows -> 8 cores x 512 rows.
"""

import os
import sys

import numpy as np

try:
    import concourse.bass as bass  # noqa: F401
except ImportError:
    sys.path.insert(0, "/opt/trn_rl_repo")
    import concourse.bass as bass  # noqa: F401

import ml_dtypes  # noqa: F401

import concourse.bacc as bacc
import concourse.tile as tile
from concourse import mybir
from concourse.bass_utils import run_bass_kernel_spmd

F32 = mybir.dt.float32
BF16 = mybir.dt.bfloat16
FP16 = mybir.dt.float16
A = mybir.AluOpType
AF = mybir.ActivationFunctionType

THETA, P_FRAC = 0.3, 0.1
BSZ, SEQ = 4096, 8192
N_CORES = 8
ROWS_PER_CORE = BSZ // N_CORES          # 512
P = 128
N_TILES = ROWS_PER_CORE // P            # 4
HALF = SEQ // 2
CHUNK, CHUNK_EVERY = 8, 64              # subsample: 8 cols every 64 of half
SUB = HALF // CHUNK_EVERY * CHUNK       # 512
BIG = 1.0e30
HH = 1.0e25                             # relu count scaling
MASKVAL = -60000.0                      # fp16-representable, exp -> 0
SM_THRESH = -1000.0                     # finite iff sm > this

N_SUB = int(os.environ.get("NORM_SUB_ITERS", "5"))
FULL_SEQ = os.environ.get("NORM_FULL_SEQ", "FSS")  # F=fixed point, S=secant
# column-slice widths (DVE vs ACT) for the full s-passes and k-passes
S_DVE = int(os.environ.get("NORM_S_DVE", "3712"))
S_ACT = SEQ - S_DVE
K_DVE = int(os.environ.get("NORM_K_DVE", "4096"))
K_ACT = SEQ - K_DVE
DEBUG = os.environ.get("NORM_DEBUG", "0") == "1"


def _sub_view(ap):
    """[P, SEQ] AP -> [P, 64, CHUNK] subsample view of the first half
    (columns are iid; first-half-only sampling lets the sub phase start
    before the second half-DMAs land)."""
    return ap[:, 0:HALF].rearrange(
        "p (c l) -> p c l", l=CHUNK_EVERY)[:, :, 0:CHUNK]


def _sub_out(ap):
    """[P, SUB] contiguous AP -> [P, 64, CHUNK] view."""
    return ap.rearrange("p (c l) -> p c l", l=CHUNK)


def build_kernel():
    nc = bacc.Bacc("TRN2", target_bir_lowering=False, debug=False,
                   num_devices=N_CORES)
    sm_d = nc.dram_tensor("sm", [ROWS_PER_CORE, SEQ], FP16,
                          kind="ExternalInput")
    gamma_d = nc.dram_tensor("gamma", [ROWS_PER_CORE, SEQ], BF16,
                             kind="ExternalOutput")

    v = nc.vector
    g = nc.gpsimd
    s = nc.scalar

    with tile.TileContext(nc) as tc:
        with (
            tc.tile_pool(name="smp", bufs=1) as smp,
            tc.tile_pool(name="ep", bufs=1) as ep,
            tc.tile_pool(name="jdp", bufs=1) as jdp,
            tc.tile_pool(name="jap", bufs=1) as jap,
            tc.tile_pool(name="jsp", bufs=1) as jsp,
            tc.tile_pool(name="scal", bufs=8) as scal,
        ):
            jD = jdp.tile([P, max(K_DVE, S_DVE, SUB)], F32, tag="jD")
            jA = jap.tile([P, max(K_ACT, S_ACT)], F32, tag="jA")
            jSD = jsp.tile([P, SUB], F32, tag="jSD")
            jSA = jsp.tile([P, SUB], F32, tag="jSA")

            cnts4 = scal.tile([P, N_TILES], F32, tag="cnts")
            kD4 = scal.tile([P, N_TILES], F32, tag="kD")
            rkA4 = scal.tile([P, N_TILES], F32, tag="rkA")
            cSUB = scal.tile([P, 2], F32, tag="cSUB")
            g.memset(cSUB[:], float(SUB))
            eps30 = scal.tile([P, N_TILES], F32, tag="eps30")
            g.memset(eps30[:], 1e-30)

            # ---- phase A: DMA in (first halves first), exp (h1-first),
            # DVE count slices --------------------------------------------
            E, sms = [], []
            for j in range(N_TILES):
                r0 = j * P
                sm = smp.tile([P, SEQ], FP16, tag=f"sm{j}")
                sms.append(sm)
                nc.sync.dma_start(out=sm[:][:, 0:HALF],
                                  in_=sm_d.ap()[r0:r0 + P, 0:HALF])
                e_t = ep.tile([P, SEQ], BF16, tag=f"E{j}")
                E.append(e_t)
            for j in range(N_TILES):
                r0 = j * P
                nc.sync.dma_start(out=sms[j][:][:, HALF:SEQ],
                                  in_=sm_d.ap()[r0:r0 + P, HALF:SEQ])
            for j in range(N_TILES):
                s.activation(out=E[j][:][:, 0:HALF],
                             in_=sms[j][:][:, 0:HALF],
                             func=AF.Exp, scale=1.0 / THETA)
                # count slices read only the first half of sm (K_DVE=4096)
                v.tensor_scalar(out=jD[:][:, 0:K_DVE],
                                in0=sms[j][:][:, 0:K_DVE],
                                scalar1=SM_THRESH, scalar2=None,
                                op0=A.is_gt, op1=A.add,
                                accum_out=kD4[:, j:j + 1])
                v.tensor_scalar(out=_sub_out(jSD[:]),
                                in0=_sub_view(sms[j][:]),
                                scalar1=SM_THRESH, scalar2=None,
                                op0=A.is_gt, op1=A.add,
                                accum_out=cnts4[:, j:j + 1])
            for j in range(N_TILES):
                s.activation(out=E[j][:][:, HALF:SEQ],
                             in_=sms[j][:][:, HALF:SEQ],
                             func=AF.Exp, scale=1.0 / THETA)

            # per-group subsample rks = 10 / cnt_sub
            rks = []
            for grp in range(2):
                rc_ = scal.tile([P, 2], F32, tag=f"rcs{grp}")
                v.reciprocal(rc_[:], cnts4[:, 2 * grp:2 * grp + 2])
                rk_ = scal.tile([P, 2], F32, tag=f"rks{grp}")
                v.tensor_scalar_mul(rk_[:], rc_[:], 1.0 / P_FRAC)
                rks.append(rk_)

            # ---- phase B: subsample fixed point, 2 chains ---------------
            # group A = tiles {0,1} on DVE, group B = tiles {2,3} on ACT
            cA = cB = None
            for it in range(N_SUB):
                sA = scal.tile([P, 2], F32, tag="sgA")
                rB = scal.tile([P, 2], F32, tag="sgB")
                for jj in range(2):
                    v.tensor_scalar(out=_sub_out(jSD[:]),
                                    in0=_sub_view(E[jj][:]),
                                    scalar1=(BIG if it == 0
                                             else cA[:, jj:jj + 1]),
                                    scalar2=None,
                                    op0=A.min, op1=A.add,
                                    accum_out=sA[:, jj:jj + 1])
                for jj in range(2):
                    if it == 0:
                        s.activation(out=_sub_out(jSA[:]),
                                     in_=_sub_view(E[2 + jj][:]),
                                     func=AF.Identity,
                                     accum_out=rB[:, jj:jj + 1])
                    else:
                        s.activation(out=_sub_out(jSA[:]),
                                     in_=_sub_view(E[2 + jj][:]),
                                     func=AF.Relu, scale=-1.0,
                                     bias=cB[:, jj:jj + 1],
                                     accum_out=rB[:, jj:jj + 1])
                cAn = scal.tile([P, 2], F32, tag="cgA")
                v.tensor_mul(cAn[:], sA[:], rks[0][:])
                cBn = scal.tile([P, 2], F32, tag="cgB")
                if it == 0:
                    g.tensor_mul(cBn[:], rB[:], rks[1][:])
                else:
                    # s = SUB*c - r ; c' = s * rks   (all on Pool)
                    uB = scal.tile([P, 2], F32, tag="ugB")
                    g.tensor_mul(uB[:], cB[:], cSUB[:])
                    tB = scal.tile([P, 2], F32, tag="tgB")
                    g.tensor_sub(tB[:], uB[:], rB[:])
                    g.tensor_mul(cBn[:], tB[:], rks[1][:])
                cA, cB = cAn, cBn

            # merge group c into batched [P,4]
            c4 = scal.tile([P, N_TILES], F32, tag="c4m")
            v.tensor_copy(c4[:, 0:2], cA[:])
            v.tensor_copy(c4[:, 2:4], cB[:])

            # ---- ACT count slices (fill ACT gaps) + k prep --------------
            for j in range(N_TILES):
                s.activation(out=jA[:][:, 0:K_ACT],
                             in_=E[j][:][:, K_DVE:SEQ],
                             func=AF.Relu, scale=-HH, bias=1.0,
                             accum_out=rkA4[:, j:j + 1])
            t1 = scal.tile([P, N_TILES], F32, tag="t1")
            v.scalar_tensor_tensor(out=t1[:], in0=rkA4[:], scalar=-1.0,
                                   in1=kD4[:], op0=A.mult, op1=A.add)
            cnt4 = scal.tile([P, N_TILES], F32, tag="cnt4")
            v.tensor_scalar_add(cnt4[:], t1[:], float(K_ACT))
            k4 = scal.tile([P, N_TILES], F32, tag="k4")
            v.tensor_scalar_mul(k4[:], cnt4[:], P_FRAC)
            rk4 = scal.tile([P, N_TILES], F32, tag="rk4")
            v.reciprocal(rk4[:], k4[:])
            k02 = scal.tile([P, N_TILES], F32, tag="k02")
            v.tensor_scalar_mul(k02[:], k4[:], 0.02)

            # ---- phase C: full-width passes (FSS) -----------------------
            def full_s_pass(c_t, tag):
                sD = scal.tile([P, N_TILES], F32, tag="sD" + tag)
                rA = scal.tile([P, N_TILES], F32, tag="rA" + tag)
                for j in range(N_TILES):
                    cj = c_t[:, j:j + 1]
                    v.tensor_scalar(out=jD[:][:, 0:S_DVE],
                                    in0=E[j][:][:, 0:S_DVE],
                                    scalar1=cj, scalar2=None,
                                    op0=A.min, op1=A.add,
                                    accum_out=sD[:, j:j + 1])
                    s.activation(out=jA[:][:, 0:S_ACT],
                                 in_=E[j][:][:, S_DVE:SEQ],
                                 func=AF.Relu, scale=-1.0, bias=cj,
                                 accum_out=rA[:, j:j + 1])
                # s = sD + S_ACT*c - rA
                u1 = scal.tile([P, N_TILES], F32, tag="u1" + tag)
                v.scalar_tensor_tensor(out=u1[:], in0=c_t[:],
                                       scalar=float(S_ACT), in1=rA[:],
                                       op0=A.mult, op1=A.subtract)
                s4 = scal.tile([P, N_TILES], F32, tag="s4" + tag)
                v.tensor_add(s4[:], sD[:], u1[:])
                return s4

            cp, sp_ = None, None
            for i, stepc in enumerate(FULL_SEQ):
                if stepc != "F":
                    # c-only secant inputs: run during the s-pass
                    dc = scal.tile([P, N_TILES], F32, tag=f"dc{i}")
                    g.tensor_sub(dc[:], c4[:], cp[:])
                    ec = scal.tile([P, N_TILES], F32, tag=f"ec{i}")
                    g.tensor_mul(ec[:], c4[:], eps30[:])
                    dc2 = scal.tile([P, N_TILES], F32, tag=f"dc2{i}")
                    g.tensor_add(dc2[:], dc[:], ec[:])
                    kc = scal.tile([P, N_TILES], F32, tag=f"kc{i}")
                    g.tensor_mul(kc[:], k4[:], c4[:])
                    rdc = scal.tile([P, N_TILES], F32, tag=f"rdc{i}")
                    v.reciprocal(rdc[:], dc2[:])
                s4 = full_s_pass(c4, f"f{i}")
                cn = scal.tile([P, N_TILES], F32, tag=f"c4_{i}")
                if stepc == "F":
                    v.tensor_mul(cn[:], s4[:], rk4[:])
                else:  # secant, post-pass chain
                    ds = scal.tile([P, N_TILES], F32, tag=f"ds{i}")
                    v.tensor_sub(ds[:], s4[:], sp_[:])
                    m_ = scal.tile([P, N_TILES], F32, tag=f"m{i}")
                    v.tensor_mul(m_[:], ds[:], rdc[:])
                    den = scal.tile([P, N_TILES], F32, tag=f"den{i}")
                    v.tensor_sub(den[:], k4[:], m_[:])
                    den2 = scal.tile([P, N_TILES], F32, tag=f"den2{i}")
                    v.tensor_max(den2[:], den[:], k02[:])
                    rden = scal.tile([P, N_TILES], F32, tag=f"rden{i}")
                    v.reciprocal(rden[:], den2[:])
                    num = scal.tile([P, N_TILES], F32, tag=f"num{i}")
                    v.tensor_sub(num[:], s4[:], kc[:])
                    tq = scal.tile([P, N_TILES], F32, tag=f"tq{i}")
                    v.tensor_mul(tq[:], num[:], rden[:])
                    v.tensor_add(cn[:], c4[:], tq[:])
                cp, sp_ = c4, s4
                c4 = cn

            # ---- phase D: gamma (in place over E, halves), DMA out ------
            rc4 = scal.tile([P, N_TILES], F32, tag="rc4")
            v.reciprocal(rc4[:], c4[:])
            for j in range(N_TILES):
                r0 = j * P
                for h0, h1 in ((0, HALF), (HALF, SEQ)):
                    v.tensor_scalar(out=E[j][:][:, h0:h1],
                                    in0=E[j][:][:, h0:h1],
                                    scalar1=rc4[:, j:j + 1], scalar2=1.0,
                                    op0=A.mult, op1=A.min)
                    nc.sync.dma_start(out=gamma_d.ap()[r0:r0 + P, h0:h1],
                                      in_=E[j][:][:, h0:h1])

    nc.compile()
    return nc


_NC_CACHE = None


def prep_sm(score: np.ndarray, mask: np.ndarray) -> np.ndarray:
    """host-side dtype prep: masked score in fp16 (elementwise only)."""
    return np.where(np.asarray(mask) == 0, np.float16(MASKVAL),
                    np.asarray(score).astype(np.float16))


def kernel(score: np.ndarray, mask: np.ndarray) -> np.ndarray:
    global _NC_CACHE
    if _NC_CACHE is None:
        _NC_CACHE = build_kernel()
    nc = _NC_CACHE

    sm16 = np.ascontiguousarray(prep_sm(score, mask))
    in_maps = []
    for i in range(N_CORES):
        sl = slice(i * ROWS_PER_CORE, (i + 1) * ROWS_PER_CORE)
        in_maps.append({"sm": sm16[sl]})
    res = run_bass_kernel_spmd(nc, in_maps, core_ids=list(range(N_CORES)))
    out = np.concatenate([res.results[i]["gamma"] for i in range(N_CORES)],
                         axis=0)
    return out.astype(np.float32)
